# revision 5
# baseline (speedup 1.0000x reference)
"""Trainium2 Bass kernel for nn_HadamardClassifier (self-contained).

Math: out = -scale * l2norm_rows(x) @ H + bias, with H the [2048, 14951]
top-left slice of the 16384x16384 Sylvester Hadamard matrix,
H[i, j] = (-1)^popcount(i & j).

Since row index i < 2048 uses only 11 bits, H[i, j] == H2048[i, j & 2047]:
the output is a periodic tiling of y = xn' @ H2048 (7.3x FLOP reduction).
Further, H2048 = H16 (x) H128 (Kronecker split at bit 7), so
    y[m, jh*128 + jl] = sum_ih H16[ih, jh] * z[m, ih, jl]
    z[m, ih, jl]      = sum_il H128[il, jl] * xn'[m, ih*128 + il]
Stage 1 is a K=128 PE matmul against H128 (with x panels PE-transposed),
stage 2 is a 4-stage FWHT over ih on the vector engine, then a PE
transpose-back, then the 7.3x fan-out with per-block bias (PE copy-matmul +
rank-1 bias matmul into PSUM, ACT evacuate; a fraction of tiles go through a
direct DVE add to balance engine load).

Sharding: data-parallel over batch, 8 cores x 512 rows. No collectives.
"""

import numpy as np

BATCH = 4096
IN_DIM = 2048
OUT_DIM = 14951
EPS = 1e-12
N_CORES = 8
M_PER_CORE = BATCH // N_CORES          # 512
N_CHUNKS = M_PER_CORE // 128           # 4 m-chunks of 128 rows
N_FULL_BLOCKS = OUT_DIM // IN_DIM      # 7
TAIL_COLS = OUT_DIM - N_FULL_BLOCKS * IN_DIM  # 615

F32 = None  # set lazily (mybir import)


def _hadamard(n):
    """Sylvester Hadamard matrix H[i,j] = (-1)^popcount(i&j), float32."""
    i = np.arange(n, dtype=np.uint32)[:, None]
    j = np.arange(n, dtype=np.uint32)[None, :]
    v = i & j
    pc = np.zeros_like(v)
    for b in range(int(n).bit_length()):
        pc += (v >> b) & 1
    return (1.0 - 2.0 * (pc & 1)).astype(np.float32)


def _patch_tile_drain():
    """This walrus build accepts only ONE sync-wait per instruction, but
    Tile's kernel-tail drain attaches the whole global clock to a single
    Drain ('Too many sync wait commands').  Split the waits onto a chain of
    single-wait sequencer nops instead."""
    import concourse.mybir as mybir
    import concourse.tile as tile
    from concourse.vector_clock import ScopedClock

    if getattr(tile.TileContext, "_drain_split_patched", False):
        return

    def _drain_and_barrier(self, tick_clock, wait_clock):
        nc = self.nc
        probe = nc.sync.nop()
        wait_clock.add_sem_waits(
            probe.ins, ScopedClock({None: tick_clock.global_clock})
        )
        si = probe.ins.sync_info
        waits = list(si.on_wait) if si is not None and si.on_wait else []
        if len(waits) > 1:
            si.on_wait = waits[:1]
            for w in waits[1:]:
                n = nc.sync.nop()
                n.ins.sync_info = mybir.SyncInfo(on_wait=[w], on_update=[])
        nc.sync.drain()
        nc.all_engine_barrier()
        assert self.sems is not None
        popped = nc._tile_sem_poison_stack.pop()
        assert popped is self._sem_poison
        nc.clear_and_free_semaphores(list(self.sems.allocated().values()))
        nc.all_engine_barrier()

    tile.TileContext._drain_and_barrier = _drain_and_barrier
    tile.TileContext._drain_split_patched = True


def _split_multiwait_instructions(nc):
    """This walrus build rejects instructions with more than one sync-wait.
    Hoist extra waits onto same-engine nop instructions inserted just before
    the offending instruction (engine queues execute in order, so waiting on
    the nops first is equivalent)."""
    import concourse.mybir as mybir

    n_split = 0
    for blk in nc.m.functions[0].blocks:
        new_list = []
        for inst in blk.instructions:
            si = inst.sync_info
            waits = list(si.on_wait) if si is not None and si.on_wait else []
            if len(waits) > 1:
                for k, w in enumerate(waits[:-1]):
                    nop = mybir.InstNoOp(
                        name=f"{inst.name}-wsplit{k}", ins=[], outs=[])
                    nop.engine = inst.engine
                    nop.sync_info = mybir.SyncInfo(on_wait=[w], on_update=[])
                    new_list.append(nop)
                    n_split += 1
                si.on_wait = waits[-1:]
            new_list.append(inst)
        blk.instructions = new_list
    return n_split


def _build_program():
    import concourse.bass as bass
    import concourse.mybir as mybir
    import concourse.tile as tile

    _patch_tile_drain()
    f32 = mybir.dt.float32
    nc = bass.Bass()

    x_d = nc.dram_tensor("x", [M_PER_CORE, IN_DIM], f32, kind="ExternalInput")
    h128_d = nc.dram_tensor("h128", [128, 128], f32, kind="ExternalInput")
    ident_d = nc.dram_tensor("ident", [128, 128], f32, kind="ExternalInput")
    ones_d = nc.dram_tensor("ones", [1, 128], f32, kind="ExternalInput")
    bias_d = nc.dram_tensor("bias", [OUT_DIM], f32, kind="ExternalInput")
    nscale_d = nc.dram_tensor("nscale", [128, 1], f32, kind="ExternalInput")
    out_d = nc.dram_tensor("out", [M_PER_CORE, OUT_DIM], f32, kind="ExternalOutput")

    from contextlib import ExitStack

    with tile.TileContext(nc) as tc, ExitStack() as ctx:
        singles = ctx.enter_context(tc.tile_pool(name="singles", bufs=1))
        xpool = ctx.enter_context(tc.tile_pool(name="xpool", bufs=2))
        scrpool = ctx.enter_context(tc.tile_pool(name="scr", bufs=2))
        xtpool = ctx.enter_context(tc.tile_pool(name="xt", bufs=2))
        fwpool = ctx.enter_context(tc.tile_pool(name="fw", bufs=2))
        ypool = ctx.enter_context(tc.tile_pool(name="y", bufs=2))
        outpool = ctx.enter_context(tc.tile_pool(name="outp", bufs=3))
        tp_ps = ctx.enter_context(tc.tile_pool(name="tp_ps", bufs=2, space="PSUM"))
        z_ps = ctx.enter_context(tc.tile_pool(name="z_ps", bufs=2, space="PSUM"))
        fan_ps = ctx.enter_context(tc.tile_pool(name="fan_ps", bufs=4, space="PSUM"))

        # --- constants ---
        h128_s = singles.tile([128, 128], f32)
        nc.sync.dma_start(out=h128_s, in_=h128_d[:, :])
        ident_s = singles.tile([128, 128], f32)
        nc.sync.dma_start(out=ident_s, in_=ident_d[:, :])
        ones_s = singles.tile([1, 128], f32)
        nc.sync.dma_start(out=ones_s, in_=ones_d[:, :])
        nscale_s = singles.tile([128, 1], f32)
        nc.sync.dma_start(out=nscale_s, in_=nscale_d[:, :])
        eps_s = singles.tile([128, 1], f32)
        nc.vector.memset(eps_s, EPS)
        # bias broadcast to all 128 partitions: [128, OUT_DIM]
        bias_b = singles.tile([128, OUT_DIM], f32)
        bias_ap = bias_d[:]
        bias_bcast_src = bass.AP(
            tensor=bias_ap.tensor, offset=bias_ap.offset,
            ap=[[0, 128]] + list(bias_ap.ap),
        )
        nc.gpsimd.dma_start(out=bias_b, in_=bias_bcast_src)

        for c in range(N_CHUNKS):
            rows = slice(c * 128, (c + 1) * 128)

            # --- load x chunk ---
            x_c = xpool.tile([128, IN_DIM], f32)
            nc.gpsimd.dma_start(out=x_c, in_=x_d[rows, :])

            # --- row norms: ss = sum(x^2), rs2 = -scale / sqrt(ss + eps) ---
            sq = scrpool.tile([128, 1024], f32, tag="sq")
            ss0 = scrpool.tile([128, 1], f32, tag="ss0")
            ss1 = scrpool.tile([128, 1], f32, tag="ss1")
            nc.scalar.activation(
                out=sq, in_=x_c[:, :1024],
                func=mybir.ActivationFunctionType.Square, accum_out=ss0)
            nc.scalar.activation(
                out=sq, in_=x_c[:, 1024:],
                func=mybir.ActivationFunctionType.Square, accum_out=ss1)
            rs = scrpool.tile([128, 1], f32, tag="rs")
            nc.vector.tensor_add(out=rs, in0=ss0, in1=ss1)
            nc.scalar.activation(
                out=rs, in_=rs, func=mybir.ActivationFunctionType.Sqrt,
                bias=eps_s)
            nc.vector.reciprocal(out=rs, in_=rs)
            nc.vector.tensor_mul(out=rs, in0=rs, in1=nscale_s)
            # normalize in place: x_c = x_c * rs2 (per-partition scalar)
            nc.scalar.activation(
                out=x_c, in_=x_c, func=mybir.ActivationFunctionType.Copy,
                scale=rs)

            # --- PE transpose x panels: xnT[il, ih, m] = xn'[m, ih*128+il] ---
            xnT = xtpool.tile([128, 16, 128], f32)
            for g in range(4):
                tp = tp_ps.tile([128, 512], f32, tag="tp")
                for hh in range(4):
                    h = 4 * g + hh
                    nc.tensor.transpose(
                        tp[:, hh * 128:(hh + 1) * 128],
                        x_c[:, h * 128:(h + 1) * 128],
                        ident_s)
                nc.scalar.copy(out=xnT[:, 4 * g:4 * g + 4, :], in_=tp)

            # --- stage 1: z[jl, ih, m] = sum_il H128[il, jl] xnT[il, ih, m] ---
            zw0 = fwpool.tile([128, 16, 128], f32, tag="zw0")
            zw1 = fwpool.tile([128, 16, 128], f32, tag="zw1")
            for g in range(4):
                zp = z_ps.tile([128, 512], f32, tag="zp")
                nc.tensor.matmul(
                    zp, lhsT=h128_s, rhs=xnT[:, 4 * g:4 * g + 4, :],
                    start=True, stop=True)
                nc.scalar.copy(out=zw0[:, 4 * g:4 * g + 4, :], in_=zp)

            # --- stage 2: FWHT over ih (dim 1), 4 butterfly stages on DVE ---
            cur, nxt = zw0, zw1
            for s in range(4):
                t = 1 << s
                cv = cur.rearrange("p (g two t) m -> p g two t m", two=2, t=t)
                nv = nxt.rearrange("p (g two t) m -> p g two t m", two=2, t=t)
                nc.vector.tensor_add(
                    out=nv[:, :, 0], in0=cv[:, :, 0], in1=cv[:, :, 1])
                nc.vector.tensor_tensor(
                    nv[:, :, 1], cv[:, :, 0], cv[:, :, 1],
                    mybir.AluOpType.subtract)
                cur, nxt = nxt, cur
            # after 4 stages result is back in zw0 (== cur)

            # --- transpose back: y[m, jh, jl] ---
            y_sb = ypool.tile([128, 16, 128], f32)
            for g in range(4):
                tp = tp_ps.tile([128, 512], f32, tag="tp")
                for hh in range(4):
                    h = 4 * g + hh
                    nc.tensor.transpose(
                        tp[:, hh * 128:(hh + 1) * 128],
                        cur[:, h, :],
                        ident_s)
                nc.scalar.copy(out=y_sb[:, 4 * g:4 * g + 4, :], in_=tp)
            y_flat = y_sb.rearrange("p a b -> p (a b)")

            # --- fan-out: out[m, 2048*b + r] = y[m, r] + bias[2048*b + r] ---
            for b in range(N_FULL_BLOCKS + 1):
                ncols = IN_DIM if b < N_FULL_BLOCKS else TAIL_COLS
                nq = (ncols + 511) // 512
                out_sb = outpool.tile([128, IN_DIM], f32)
                pe_tiles = []
                for q in range(nq):
                    w = min(512, ncols - q * 512)
                    r0 = q * 512
                    j0 = b * IN_DIM + r0
                    if q == nq - 1 and b % 2 == 1:
                        # DVE-direct path (balance engines)
                        nc.vector.tensor_add(
                            out=out_sb[:, r0:r0 + w],
                            in0=y_flat[:, r0:r0 + w],
                            in1=bias_b[:, j0:j0 + w])
                    else:
                        fp = fan_ps.tile([128, 512], f32, tag="fan")
                        nc.tensor.matmul(
                            fp[:, :w], lhsT=ident_s, rhs=y_flat[:, r0:r0 + w],
                            start=True, stop=False)
                        pe_tiles.append((fp, q, w, j0))
                for fp, q, w, j0 in pe_tiles:
                    nc.tensor.matmul(
                        fp[:, :w], lhsT=ones_s[0:1, :],
                        rhs=bias_b[0:1, j0:j0 + w],
                        start=False, stop=True)
                for fp, q, w, j0 in pe_tiles:
                    nc.scalar.copy(out=out_sb[:, q * 512:q * 512 + w],
                                   in_=fp[:, :w])
                nc.sync.dma_start(
                    out=out_d[rows, b * IN_DIM:b * IN_DIM + ncols],
                    in_=out_sb[:, :ncols])

    _split_multiwait_instructions(nc)
    return nc


_PROGRAM = None


def _get_program():
    global _PROGRAM
    if _PROGRAM is None:
        _PROGRAM = _build_program()
    return _PROGRAM


def _run(inputs, trace=False, tmpdir=None):
    from concourse.bass_utils import run_bass_kernel_spmd

    x = np.ascontiguousarray(np.asarray(inputs["x"], dtype=np.float32))
    scale = np.asarray(inputs["scale"], dtype=np.float32)
    bias = np.ascontiguousarray(np.asarray(inputs["bias"], dtype=np.float32))
    assert x.shape == (BATCH, IN_DIM) and bias.shape == (OUT_DIM,)

    h128 = _hadamard(128)
    ident = np.eye(128, dtype=np.float32)
    ones = np.ones((1, 128), dtype=np.float32)
    nscale = np.full((128, 1), -float(scale.reshape(-1)[0]), dtype=np.float32)

    shards = x.reshape(N_CORES, M_PER_CORE, IN_DIM)
    in_maps = [
        {
            "x": np.ascontiguousarray(shards[i]),
            "h128": h128,
            "ident": ident,
            "ones": ones,
            "bias": bias,
            "nscale": nscale,
        }
        for i in range(N_CORES)
    ]
    nc = _get_program()
    res = run_bass_kernel_spmd(
        nc, in_maps, core_ids=list(range(N_CORES)), trace=trace, tmpdir=tmpdir
    )
    out = np.concatenate([r["out"] for r in res.results], axis=0)
    return out, res


def kernel(x, scale, bias):
    out, _ = _run({"x": x, "scale": scale, "bias": bias})
    return out


# revision 8
# speedup vs baseline: 1.9817x; 1.9817x over previous
"""Trainium2 Bass kernel for nn_HadamardClassifier (self-contained).

Math: out = -scale * l2norm_rows(x) @ H + bias, with H the [2048, 14951]
top-left slice of the 16384x16384 Sylvester Hadamard matrix,
H[i, j] = (-1)^popcount(i & j).

Since row index i < 2048 uses only 11 bits, H[i, j] == H2048[i, j & 2047]:
the output is a periodic tiling of y = xn' @ H2048 (7.3x FLOP reduction).
Further, H2048 = H16 (x) H128 (Kronecker split at bit 7), so
    y[m, jh*128 + jl] = sum_ih H16[ih, jh] * z[m, ih, jl]
    z[m, ih, jl]      = sum_il H128[il, jl] * xn'[m, ih*128 + il]
Stage 1 is a K=128 PE matmul against H128 (with x panels PE-transposed),
stage 2 is a 4-stage FWHT over ih on the vector engine, then a PE
transpose-back, then the 7.3x fan-out with per-block bias (PE copy-matmul +
rank-1 bias matmul into PSUM, ACT evacuate; a fraction of tiles go through a
direct DVE add to balance engine load).

Sharding: data-parallel over batch, 8 cores x 512 rows. No collectives.
"""

import numpy as np

BATCH = 4096
IN_DIM = 2048
OUT_DIM = 14951
EPS = 1e-12
N_CORES = 8
M_PER_CORE = BATCH // N_CORES          # 512
N_CHUNKS = M_PER_CORE // 128           # 4 m-chunks of 128 rows
N_FULL_BLOCKS = OUT_DIM // IN_DIM      # 7
TAIL_COLS = OUT_DIM - N_FULL_BLOCKS * IN_DIM  # 615

F32 = None  # set lazily (mybir import)


def _hadamard(n):
    """Sylvester Hadamard matrix H[i,j] = (-1)^popcount(i&j), float32."""
    i = np.arange(n, dtype=np.uint32)[:, None]
    j = np.arange(n, dtype=np.uint32)[None, :]
    v = i & j
    pc = np.zeros_like(v)
    for b in range(int(n).bit_length()):
        pc += (v >> b) & 1
    return (1.0 - 2.0 * (pc & 1)).astype(np.float32)


def _patch_tile_drain():
    """This walrus build accepts only ONE sync-wait per instruction, but
    Tile's kernel-tail drain attaches the whole global clock to a single
    Drain ('Too many sync wait commands').  Split the waits onto a chain of
    single-wait sequencer nops instead."""
    import concourse.mybir as mybir
    import concourse.tile as tile
    from concourse.vector_clock import ScopedClock

    if getattr(tile.TileContext, "_drain_split_patched", False):
        return

    def _drain_and_barrier(self, tick_clock, wait_clock):
        nc = self.nc
        probe = nc.sync.nop()
        wait_clock.add_sem_waits(
            probe.ins, ScopedClock({None: tick_clock.global_clock})
        )
        si = probe.ins.sync_info
        waits = list(si.on_wait) if si is not None and si.on_wait else []
        if len(waits) > 1:
            si.on_wait = waits[:1]
            for w in waits[1:]:
                n = nc.sync.nop()
                n.ins.sync_info = mybir.SyncInfo(on_wait=[w], on_update=[])
        nc.sync.drain()
        nc.all_engine_barrier()
        assert self.sems is not None
        popped = nc._tile_sem_poison_stack.pop()
        assert popped is self._sem_poison
        nc.clear_and_free_semaphores(list(self.sems.allocated().values()))
        nc.all_engine_barrier()

    tile.TileContext._drain_and_barrier = _drain_and_barrier
    tile.TileContext._drain_split_patched = True


def _split_multiwait_instructions(nc):
    """This walrus build rejects instructions with more than one sync-wait.
    Hoist extra waits onto same-engine nop instructions inserted just before
    the offending instruction (engine queues execute in order, so waiting on
    the nops first is equivalent)."""
    import concourse.mybir as mybir

    n_split = 0
    for blk in nc.m.functions[0].blocks:
        new_list = []
        for inst in blk.instructions:
            si = inst.sync_info
            waits = list(si.on_wait) if si is not None and si.on_wait else []
            if len(waits) > 1:
                for k, w in enumerate(waits[:-1]):
                    nop = mybir.InstNoOp(
                        name=f"{inst.name}-wsplit{k}", ins=[], outs=[])
                    nop.engine = inst.engine
                    nop.sync_info = mybir.SyncInfo(on_wait=[w], on_update=[])
                    new_list.append(nop)
                    n_split += 1
                si.on_wait = waits[-1:]
            new_list.append(inst)
        blk.instructions = new_list
    return n_split


def _build_program():
    import concourse.bass as bass
    import concourse.mybir as mybir
    import concourse.tile as tile

    _patch_tile_drain()
    f32 = mybir.dt.float32
    nc = bass.Bass()

    x_d = nc.dram_tensor("x", [M_PER_CORE, IN_DIM], f32, kind="ExternalInput")
    h128_d = nc.dram_tensor("h128", [128, 128], f32, kind="ExternalInput")
    ident_d = nc.dram_tensor("ident", [128, 128], f32, kind="ExternalInput")
    bias_d = nc.dram_tensor("bias", [OUT_DIM], f32, kind="ExternalInput")
    nscale_d = nc.dram_tensor("nscale", [128, 1], f32, kind="ExternalInput")
    out_d = nc.dram_tensor("out", [M_PER_CORE, OUT_DIM], f32, kind="ExternalOutput")

    from contextlib import ExitStack

    with tile.TileContext(nc) as tc, ExitStack() as ctx:
        singles = ctx.enter_context(tc.tile_pool(name="singles", bufs=1))
        xpool = ctx.enter_context(tc.tile_pool(name="xpool", bufs=2))
        scrpool = ctx.enter_context(tc.tile_pool(name="scr", bufs=2))
        xtpool = ctx.enter_context(tc.tile_pool(name="xt", bufs=2))
        fwpool = ctx.enter_context(tc.tile_pool(name="fw", bufs=2))
        outpool = ctx.enter_context(tc.tile_pool(name="outp", bufs=3))
        tp_ps = ctx.enter_context(tc.tile_pool(name="tp_ps", bufs=2, space="PSUM"))
        z_ps = ctx.enter_context(tc.tile_pool(name="z_ps", bufs=2, space="PSUM"))

        # --- constants ---
        h128_s = singles.tile([128, 128], f32)
        nc.sync.dma_start(out=h128_s, in_=h128_d[:, :])
        ident_s = singles.tile([128, 128], f32)
        nc.sync.dma_start(out=ident_s, in_=ident_d[:, :])
        nscale_s = singles.tile([128, 1], f32)
        nc.sync.dma_start(out=nscale_s, in_=nscale_d[:, :])
        eps_s = singles.tile([128, 1], f32)
        nc.vector.memset(eps_s, EPS)
        # bias broadcast to all 128 partitions: [128, OUT_DIM]
        bias_b = singles.tile([128, OUT_DIM], f32)
        bias_ap = bias_d[:]
        bias_bcast_src = bass.AP(
            tensor=bias_ap.tensor, offset=bias_ap.offset,
            ap=[[0, 128]] + list(bias_ap.ap),
        )
        nc.gpsimd.dma_start(out=bias_b, in_=bias_bcast_src)

        for c in range(N_CHUNKS):
            rows = slice(c * 128, (c + 1) * 128)

            # --- load x chunk ---
            x_c = xpool.tile([128, IN_DIM], f32)
            nc.gpsimd.dma_start(out=x_c, in_=x_d[rows, :])

            # --- row norms: ss = sum(x^2), rs2 = -scale / sqrt(ss + eps) ---
            sq = scrpool.tile([128, 1024], f32, tag="sq")
            ss0 = scrpool.tile([128, 1], f32, tag="ss0")
            ss1 = scrpool.tile([128, 1], f32, tag="ss1")
            nc.scalar.activation(
                out=sq, in_=x_c[:, :1024],
                func=mybir.ActivationFunctionType.Square, accum_out=ss0)
            nc.scalar.activation(
                out=sq, in_=x_c[:, 1024:],
                func=mybir.ActivationFunctionType.Square, accum_out=ss1)
            rs = scrpool.tile([128, 1], f32, tag="rs")
            nc.vector.tensor_add(out=rs, in0=ss0, in1=ss1)
            nc.scalar.activation(
                out=rs, in_=rs, func=mybir.ActivationFunctionType.Sqrt,
                bias=eps_s)
            nc.vector.reciprocal(out=rs, in_=rs)
            nc.vector.tensor_mul(out=rs, in0=rs, in1=nscale_s)
            # normalize in place: x_c = x_c * rs2 (per-partition scalar)
            nc.scalar.activation(
                out=x_c, in_=x_c, func=mybir.ActivationFunctionType.Copy,
                scale=rs)

            # --- PE transpose x panels: xnT[il, ih, m] = xn'[m, ih*128+il] ---
            xnT = xtpool.tile([128, 16, 128], f32)
            for g in range(4):
                tp = tp_ps.tile([128, 512], f32, tag="tp")
                for hh in range(4):
                    h = 4 * g + hh
                    nc.tensor.transpose(
                        tp[:, hh * 128:(hh + 1) * 128],
                        x_c[:, h * 128:(h + 1) * 128],
                        ident_s)
                nc.scalar.copy(out=xnT[:, 4 * g:4 * g + 4, :], in_=tp)

            # --- stage 1: z[m, ih, jl] = sum_il xnT[il, ih, m] H128[il, jl] ---
            # lhsT = xnT panel (m-partitioned output: no transpose-back needed)
            zw0 = fwpool.tile([128, 16, 128], f32, tag="zw0")
            zw1 = fwpool.tile([128, 16, 128], f32, tag="zw1")
            for g in range(4):
                zp = z_ps.tile([128, 512], f32, tag="zp")
                for hh in range(4):
                    nc.tensor.matmul(
                        zp[:, hh * 128:(hh + 1) * 128],
                        lhsT=xnT[:, 4 * g + hh, :], rhs=h128_s,
                        start=True, stop=True)
                nc.scalar.copy(out=zw0[:, 4 * g:4 * g + 4, :], in_=zp)

            # --- stage 2: FWHT over ih (dim 1), 4 butterfly stages on DVE ---
            cur, nxt = zw0, zw1
            for s in range(4):
                t = 1 << s
                cv = cur.rearrange("p (g two t) m -> p g two t m", two=2, t=t)
                nv = nxt.rearrange("p (g two t) m -> p g two t m", two=2, t=t)
                nc.vector.tensor_add(
                    out=nv[:, :, 0], in0=cv[:, :, 0], in1=cv[:, :, 1])
                nc.vector.tensor_tensor(
                    nv[:, :, 1], cv[:, :, 0], cv[:, :, 1],
                    mybir.AluOpType.subtract)
                cur, nxt = nxt, cur
            # after 4 stages result is back in zw0 (== cur); cur IS y[m, jh, jl]
            y_flat = cur.rearrange("p a b -> p (a b)")

            # --- fan-out: out[m, 2048*b + r] = y[m, r] + bias[2048*b + r] ---
            # pure element-wise adds, split between DVE and GpSimd
            tile_idx = 0
            for b in range(N_FULL_BLOCKS + 1):
                ncols = IN_DIM if b < N_FULL_BLOCKS else TAIL_COLS
                nq = (ncols + 511) // 512
                out_sb = outpool.tile([128, IN_DIM], f32)
                for q in range(nq):
                    w = min(512, ncols - q * 512)
                    r0 = q * 512
                    j0 = b * IN_DIM + r0
                    eng = nc.vector if tile_idx % 2 == 0 else nc.gpsimd
                    eng.tensor_add(
                        out=out_sb[:, r0:r0 + w],
                        in0=y_flat[:, r0:r0 + w],
                        in1=bias_b[:, j0:j0 + w])
                    tile_idx += 1
                nc.sync.dma_start(
                    out=out_d[rows, b * IN_DIM:b * IN_DIM + ncols],
                    in_=out_sb[:, :ncols])

    _split_multiwait_instructions(nc)
    return nc


_PROGRAM = None


def _get_program():
    global _PROGRAM
    if _PROGRAM is None:
        _PROGRAM = _build_program()
    return _PROGRAM


def _run(inputs, trace=False, tmpdir=None):
    from concourse.bass_utils import run_bass_kernel_spmd

    x = np.ascontiguousarray(np.asarray(inputs["x"], dtype=np.float32))
    scale = np.asarray(inputs["scale"], dtype=np.float32)
    bias = np.ascontiguousarray(np.asarray(inputs["bias"], dtype=np.float32))
    assert x.shape == (BATCH, IN_DIM) and bias.shape == (OUT_DIM,)

    h128 = _hadamard(128)
    ident = np.eye(128, dtype=np.float32)
    nscale = np.full((128, 1), -float(scale.reshape(-1)[0]), dtype=np.float32)

    shards = x.reshape(N_CORES, M_PER_CORE, IN_DIM)
    in_maps = [
        {
            "x": np.ascontiguousarray(shards[i]),
            "h128": h128,
            "ident": ident,
            "bias": bias,
            "nscale": nscale,
        }
        for i in range(N_CORES)
    ]
    nc = _get_program()
    res = run_bass_kernel_spmd(
        nc, in_maps, core_ids=list(range(N_CORES)), trace=trace, tmpdir=tmpdir
    )
    out = np.concatenate([r["out"] for r in res.results], axis=0)
    return out, res


def kernel(x, scale, bias):
    out, _ = _run({"x": x, "scale": scale, "bias": bias})
    return out


# revision 10
# speedup vs baseline: 2.0505x; 1.0347x over previous
"""Trainium2 Bass kernel for nn_HadamardClassifier (self-contained).

Math: out = -scale * l2norm_rows(x) @ H + bias, with H the [2048, 14951]
top-left slice of the 16384x16384 Sylvester Hadamard matrix,
H[i, j] = (-1)^popcount(i & j).

Since row index i < 2048 uses only 11 bits, H[i, j] == H2048[i, j & 2047]:
the output is a periodic tiling of y = xn' @ H2048 (7.3x FLOP reduction).
Further, H2048 = H16 (x) H128 (Kronecker split at bit 7), so
    y[m, jh*128 + jl] = sum_ih H16[ih, jh] * z[m, ih, jl]
    z[m, ih, jl]      = sum_il H128[il, jl] * xn'[m, ih*128 + il]
Stage 1 is a K=128 PE matmul against H128 (with x panels PE-transposed),
stage 2 is a 4-stage FWHT over ih on the vector engine, then a PE
transpose-back, then the 7.3x fan-out with per-block bias (PE copy-matmul +
rank-1 bias matmul into PSUM, ACT evacuate; a fraction of tiles go through a
direct DVE add to balance engine load).

Sharding: data-parallel over batch, 8 cores x 512 rows. No collectives.
"""

import numpy as np

BATCH = 4096
IN_DIM = 2048
OUT_DIM = 14951
EPS = 1e-12
N_CORES = 8
M_PER_CORE = BATCH // N_CORES          # 512
N_CHUNKS = M_PER_CORE // 128           # 4 m-chunks of 128 rows
N_FULL_BLOCKS = OUT_DIM // IN_DIM      # 7
TAIL_COLS = OUT_DIM - N_FULL_BLOCKS * IN_DIM  # 615

F32 = None  # set lazily (mybir import)


def _hadamard(n):
    """Sylvester Hadamard matrix H[i,j] = (-1)^popcount(i&j), float32."""
    i = np.arange(n, dtype=np.uint32)[:, None]
    j = np.arange(n, dtype=np.uint32)[None, :]
    v = i & j
    pc = np.zeros_like(v)
    for b in range(int(n).bit_length()):
        pc += (v >> b) & 1
    return (1.0 - 2.0 * (pc & 1)).astype(np.float32)


def _patch_tile_drain():
    """This walrus build accepts only ONE sync-wait per instruction, but
    Tile's kernel-tail drain attaches the whole global clock to a single
    Drain ('Too many sync wait commands').  Split the waits onto a chain of
    single-wait sequencer nops instead."""
    import concourse.mybir as mybir
    import concourse.tile as tile
    from concourse.vector_clock import ScopedClock

    if getattr(tile.TileContext, "_drain_split_patched", False):
        return

    def _drain_and_barrier(self, tick_clock, wait_clock):
        nc = self.nc
        probe = nc.sync.nop()
        wait_clock.add_sem_waits(
            probe.ins, ScopedClock({None: tick_clock.global_clock})
        )
        si = probe.ins.sync_info
        waits = list(si.on_wait) if si is not None and si.on_wait else []
        if len(waits) > 1:
            si.on_wait = waits[:1]
            for w in waits[1:]:
                n = nc.sync.nop()
                n.ins.sync_info = mybir.SyncInfo(on_wait=[w], on_update=[])
        nc.sync.drain()
        nc.all_engine_barrier()
        assert self.sems is not None
        popped = nc._tile_sem_poison_stack.pop()
        assert popped is self._sem_poison
        nc.clear_and_free_semaphores(list(self.sems.allocated().values()))
        nc.all_engine_barrier()

    tile.TileContext._drain_and_barrier = _drain_and_barrier
    tile.TileContext._drain_split_patched = True


def _split_multiwait_instructions(nc):
    """This walrus build rejects instructions with more than one sync-wait.
    Hoist extra waits onto same-engine nop instructions inserted just before
    the offending instruction (engine queues execute in order, so waiting on
    the nops first is equivalent)."""
    import concourse.mybir as mybir

    n_split = 0
    for blk in nc.m.functions[0].blocks:
        new_list = []
        for inst in blk.instructions:
            si = inst.sync_info
            waits = list(si.on_wait) if si is not None and si.on_wait else []
            if len(waits) > 1:
                for k, w in enumerate(waits[:-1]):
                    nop = mybir.InstNoOp(
                        name=f"{inst.name}-wsplit{k}", ins=[], outs=[])
                    nop.engine = inst.engine
                    nop.sync_info = mybir.SyncInfo(on_wait=[w], on_update=[])
                    new_list.append(nop)
                    n_split += 1
                si.on_wait = waits[-1:]
            new_list.append(inst)
        blk.instructions = new_list
    return n_split


def _build_program():
    import concourse.bass as bass
    import concourse.mybir as mybir
    import concourse.tile as tile

    _patch_tile_drain()
    f32 = mybir.dt.float32
    nc = bass.Bass()

    x_d = nc.dram_tensor("x", [M_PER_CORE, IN_DIM], f32, kind="ExternalInput")
    h128_d = nc.dram_tensor("h128", [128, 128], f32, kind="ExternalInput")
    ident_d = nc.dram_tensor("ident", [128, 128], f32, kind="ExternalInput")
    bias_d = nc.dram_tensor("bias", [OUT_DIM], f32, kind="ExternalInput")
    nscale_d = nc.dram_tensor("nscale", [128, 1], f32, kind="ExternalInput")
    out_d = nc.dram_tensor("out", [M_PER_CORE, OUT_DIM], f32, kind="ExternalOutput")

    from contextlib import ExitStack

    with tile.TileContext(nc) as tc, ExitStack() as ctx:
        singles = ctx.enter_context(tc.tile_pool(name="singles", bufs=1))
        xpool = ctx.enter_context(tc.tile_pool(name="xpool", bufs=2))
        scrpool = ctx.enter_context(tc.tile_pool(name="scr", bufs=2))
        xtpool = ctx.enter_context(tc.tile_pool(name="xt", bufs=2))
        fwpool = ctx.enter_context(tc.tile_pool(name="fw", bufs=3))
        outpool = ctx.enter_context(tc.tile_pool(name="outp", bufs=4))
        tp_ps = ctx.enter_context(tc.tile_pool(name="tp_ps", bufs=3, space="PSUM"))
        z_ps = ctx.enter_context(tc.tile_pool(name="z_ps", bufs=3, space="PSUM"))

        # --- constants ---
        h128_s = singles.tile([128, 128], f32)
        nc.sync.dma_start(out=h128_s, in_=h128_d[:, :])
        ident_s = singles.tile([128, 128], f32)
        nc.sync.dma_start(out=ident_s, in_=ident_d[:, :])
        nscale_s = singles.tile([128, 1], f32)
        nc.sync.dma_start(out=nscale_s, in_=nscale_d[:, :])
        eps_s = singles.tile([128, 1], f32)
        nc.vector.memset(eps_s, EPS)
        # bias broadcast to all 128 partitions: [128, OUT_DIM]
        bias_b = singles.tile([128, OUT_DIM], f32)
        bias_ap = bias_d[:]
        bias_bcast_src = bass.AP(
            tensor=bias_ap.tensor, offset=bias_ap.offset,
            ap=[[0, 128]] + list(bias_ap.ap),
        )
        nc.gpsimd.dma_start(out=bias_b, in_=bias_bcast_src)

        for c in range(N_CHUNKS):
            rows = slice(c * 128, (c + 1) * 128)

            # --- load x chunk ---
            x_c = xpool.tile([128, IN_DIM], f32)
            nc.gpsimd.dma_start(out=x_c, in_=x_d[rows, :])

            # --- row norms: ss = sum(x^2), rs2 = -scale / sqrt(ss + eps) ---
            sq = scrpool.tile([128, 1024], f32, tag="sq")
            ss0 = scrpool.tile([128, 1], f32, tag="ss0")
            ss1 = scrpool.tile([128, 1], f32, tag="ss1")
            nc.scalar.activation(
                out=sq, in_=x_c[:, :1024],
                func=mybir.ActivationFunctionType.Square, accum_out=ss0)
            nc.scalar.activation(
                out=sq, in_=x_c[:, 1024:],
                func=mybir.ActivationFunctionType.Square, accum_out=ss1)
            rs = scrpool.tile([128, 1], f32, tag="rs")
            nc.vector.tensor_add(out=rs, in0=ss0, in1=ss1)
            nc.scalar.activation(
                out=rs, in_=rs, func=mybir.ActivationFunctionType.Sqrt,
                bias=eps_s)
            nc.vector.reciprocal(out=rs, in_=rs)
            nc.vector.tensor_mul(out=rs, in0=rs, in1=nscale_s)
            # normalize in place: x_c = x_c * rs2 (per-partition scalar)
            nc.scalar.activation(
                out=x_c, in_=x_c, func=mybir.ActivationFunctionType.Copy,
                scale=rs)

            # --- PE transpose x panels: xnT[il, ih, m] = xn'[m, ih*128+il] ---
            xnT = xtpool.tile([128, 16, 128], f32)
            for g in range(4):
                tp = tp_ps.tile([128, 512], f32, tag="tp")
                for hh in range(4):
                    h = 4 * g + hh
                    nc.tensor.transpose(
                        tp[:, hh * 128:(hh + 1) * 128],
                        x_c[:, h * 128:(h + 1) * 128],
                        ident_s)
                nc.scalar.copy(out=xnT[:, 4 * g:4 * g + 4, :], in_=tp)

            # --- stage 1: z[m, ih, jl] = sum_il xnT[il, ih, m] H128[il, jl] ---
            # lhsT = xnT panel (m-partitioned output: no transpose-back needed)
            zw0 = fwpool.tile([128, 16, 128], f32, tag="zw0")
            zw1 = fwpool.tile([128, 16, 128], f32, tag="zw1")
            for g in range(4):
                zp = z_ps.tile([128, 512], f32, tag="zp")
                for hh in range(4):
                    nc.tensor.matmul(
                        zp[:, hh * 128:(hh + 1) * 128],
                        lhsT=xnT[:, 4 * g + hh, :], rhs=h128_s,
                        start=True, stop=True)
                nc.scalar.copy(out=zw0[:, 4 * g:4 * g + 4, :], in_=zp)

            # --- stage 2: FWHT over ih (dim 1), 4 butterfly stages on DVE ---
            cur, nxt = zw0, zw1
            for s in range(4):
                t = 1 << s
                cv = cur.rearrange("p (g two t) m -> p g two t m", two=2, t=t)
                nv = nxt.rearrange("p (g two t) m -> p g two t m", two=2, t=t)
                nc.vector.tensor_add(
                    out=nv[:, :, 0], in0=cv[:, :, 0], in1=cv[:, :, 1])
                nc.vector.tensor_tensor(
                    nv[:, :, 1], cv[:, :, 0], cv[:, :, 1],
                    mybir.AluOpType.subtract)
                cur, nxt = nxt, cur
            # after 4 stages result is back in zw0 (== cur); cur IS y[m, jh, jl]
            y_flat = cur.rearrange("p a b -> p (a b)")

            # --- fan-out: out[m, 2048*b + r] = y[m, r] + bias[2048*b + r] ---
            # one whole-block add per block (amortize per-op overhead),
            # split 2:1 between DVE and GpSimd
            for b in range(N_FULL_BLOCKS + 1):
                ncols = IN_DIM if b < N_FULL_BLOCKS else TAIL_COLS
                out_sb = outpool.tile([128, IN_DIM], f32)
                gb = c * (N_FULL_BLOCKS + 1) + b
                eng = nc.gpsimd if gb % 3 == 2 else nc.vector
                eng.tensor_add(
                    out=out_sb[:, :ncols],
                    in0=y_flat[:, :ncols],
                    in1=bias_b[:, b * IN_DIM:b * IN_DIM + ncols])
                nc.sync.dma_start(
                    out=out_d[rows, b * IN_DIM:b * IN_DIM + ncols],
                    in_=out_sb[:, :ncols])

    _split_multiwait_instructions(nc)
    return nc


_PROGRAM = None


def _get_program():
    global _PROGRAM
    if _PROGRAM is None:
        _PROGRAM = _build_program()
    return _PROGRAM


def _run(inputs, trace=False, tmpdir=None):
    from concourse.bass_utils import run_bass_kernel_spmd

    x = np.ascontiguousarray(np.asarray(inputs["x"], dtype=np.float32))
    scale = np.asarray(inputs["scale"], dtype=np.float32)
    bias = np.ascontiguousarray(np.asarray(inputs["bias"], dtype=np.float32))
    assert x.shape == (BATCH, IN_DIM) and bias.shape == (OUT_DIM,)

    h128 = _hadamard(128)
    ident = np.eye(128, dtype=np.float32)
    nscale = np.full((128, 1), -float(scale.reshape(-1)[0]), dtype=np.float32)

    shards = x.reshape(N_CORES, M_PER_CORE, IN_DIM)
    in_maps = [
        {
            "x": np.ascontiguousarray(shards[i]),
            "h128": h128,
            "ident": ident,
            "bias": bias,
            "nscale": nscale,
        }
        for i in range(N_CORES)
    ]
    nc = _get_program()
    res = run_bass_kernel_spmd(
        nc, in_maps, core_ids=list(range(N_CORES)), trace=trace, tmpdir=tmpdir
    )
    out = np.concatenate([r["out"] for r in res.results], axis=0)
    return out, res


def kernel(x, scale, bias):
    out, _ = _run({"x": x, "scale": scale, "bias": bias})
    return out
